# revision 28
# baseline (speedup 1.0000x reference)
"""BinaryLinear kernel for Trainium2 (8 NeuronCores, SPMD).

Computes y = x @ sign(W)^T + sign(b) with x:[8192,4096] f32,
W:[4096,4096] f32, b:[4096] f32.

Sharding: 2-way over tokens x 4-way over out_features (8 cores).
Per core: x_shard [4096, 4096], W_shard [1024, 4096], b_shard [1024]
-> y_shard [4096, 1024].

Math strategy: sign(W) is exactly representable in bf16 (+-1). x is
split into x = hi + lo with hi = bf16(x) and lo = bf16(x - hi);
y = hi @ sW^T + lo @ sW^T accumulated in f32 PSUM reproduces the f32
result to ~2e-6 relative error while the TensorEngine runs at bf16
rate (~78 TFLOP/s). PE work per core: 4096 LDW+MM pairs (N=512) =
~874 us; measured kernel ~1.04 ms (PE ~85% busy).

Structure per core:
  - Phase 0: sign(W)^T built resident in SBUF (8MB bf16) -- W tiles
    loaded via ACT-issued HWDGE in waves of 4, ACT Sign, then one
    [128, 4096] xbar transpose per tile on the SP queue. Bias is
    broadcast-loaded (stride-0 DMA) and signed.
  - Phase 1 (per 128-token tile): SWDGE x load, DVE hi-cast + lo-sub,
    two xbar transposes into [k, t] layout, 128 MMs into 2 PSUM banks
    (hi sweep then lo sweep per 512-wide out group), DVE bias-add
    eviction, SWDGE store.

Queue discipline: the SP HWDGE queue carries ONLY DMA transposes
(transposes occupy all 16 DMA engines -- keep their queue pure); bulk
copies go via SWDGE (gpsimd) or ACT-issued HWDGE. Concurrent
transposes on two HWDGE queues crash the device (NRT_EXEC_UNIT_
UNRECOVERABLE) -- keep all transposes on one queue.
"""

import sys

sys.path.insert(0, "/opt/trn_rl_repo")

import numpy as np

import concourse.bass as bass  # noqa: F401
import concourse.mybir as mybir
from concourse import bacc, tile
from concourse.bass_utils import run_bass_kernel_spmd

TOKENS, IN, OUT = 8192, 4096, 4096
N_CORES = 8
T_SPLIT, O_SPLIT = 2, 4
T_CORE, O_CORE = TOKENS // T_SPLIT, OUT // O_SPLIT

P = 128
FREE = 512  # matmul moving free dim / psum bank width (f32)

F32 = mybir.dt.float32
BF16 = mybir.dt.bfloat16


def emit(nc, tc, x_d, w_d, b_d, y_d, t_core, in_dim, o_core):
    """Emit the per-core program. x_d [t_core, in], w_d [o_core, in],
    b_d [1, o_core], y_d [t_core, o_core]."""
    KS = in_dim // P  # number of 128-wide k slabs
    TT = t_core // P  # token tiles
    OG = o_core // FREE  # 512-wide out groups
    OT = o_core // P  # 128-row tiles of W

    WARM = 0  # tiles that run progressive 128-wide out sweeps (see v9)

    from contextlib import ExitStack

    with ExitStack() as ctx:
        const = ctx.enter_context(tc.tile_pool(name="const", bufs=1))
        # Resident sign(W)^T: [128 k-part, KS slabs, o_core] bf16
        swt = const.tile([P, KS, o_core], BF16)
        bias_bc = const.tile([P, o_core], F32)

        # ---- Phase 0: weights + bias prep ----
        # Waves of 4 full W tiles: loads on ACT HWDGE, signs on ACT,
        # xbar transposes on the SP queue (transposes only).
        with tc.tile_pool(name="wload", bufs=4) as wpool:
            braw = wpool.tile([P, o_core], F32, name="braw", bufs=1)
            nc.gpsimd.dma_start(braw, b_d.to_broadcast([P, o_core]))
            nc.scalar.sign(bias_bc, braw)
            for w0 in range(0, OT, 4):
                wfs, wss = [], []
                for ot in range(w0, min(w0 + 4, OT)):
                    wf = wpool.tile([P, in_dim], F32, name="wf")
                    nc.scalar.dma_start(wf, w_d[ot * P : (ot + 1) * P, :])
                    wfs.append(wf)
                for wf in wfs:
                    ws = wpool.tile([P, in_dim], BF16, name="ws")
                    nc.scalar.sign(ws, wf)  # +-1 in bf16
                    wss.append(ws)
                for ot, ws in zip(range(w0, w0 + 4), wss):
                    # [128 o, in] -> [128 k, KS, 128 o]
                    nc.sync.dma_start_transpose(
                        swt[:, :, ot * P : (ot + 1) * P], ws
                    )

        # ---- Phase 1 ----
        with (
            tc.tile_pool(name="xload", bufs=2) as xpool,
            tc.tile_pool(name="hilo", bufs=2) as hpool,
            tc.tile_pool(name="xt", bufs=3) as tpool,
            tc.tile_pool(name="psum", bufs=8, space="PSUM") as psum,
            tc.tile_pool(name="yout", bufs=3) as opool,
        ):

            def prep_tile(tt):
                """x f32 load -> hi cast + lo sub (DVE) -> xbar transposes."""
                trow = slice(tt * P, (tt + 1) * P)
                xf = xpool.tile([P, in_dim], F32, name="xf")
                nc.gpsimd.dma_start(xf, x_d[trow, :])
                xhi = hpool.tile([P, in_dim], BF16, name="xhi")
                nc.vector.tensor_copy(out=xhi, in_=xf)
                xhiT = tpool.tile([P, KS, P], BF16, name="xhiT")
                nc.sync.dma_start_transpose(xhiT, xhi)
                xlo = hpool.tile([P, in_dim], BF16, name="xlo")
                nc.vector.tensor_tensor(
                    out=xlo, in0=xf, in1=xhi, op=mybir.AluOpType.subtract
                )
                xloT = tpool.tile([P, KS, P], BF16, name="xloT")
                nc.sync.dma_start_transpose(xloT, xlo)
                return xhiT, xloT

            def sweep(ps, xhiT, xloT, ocol, width):
                for ks in range(KS):
                    nc.tensor.matmul(
                        ps[:, :width], xhiT[:, ks, :], swt[:, ks, ocol],
                        start=(ks == 0), stop=False,
                    )
                for ks in range(KS):
                    nc.tensor.matmul(
                        ps[:, :width], xloT[:, ks, :], swt[:, ks, ocol],
                        start=False, stop=(ks == KS - 1),
                    )

            def mm_tile(tt, xhiT, xloT, owidth):
                """matmul sweeps in owidth-wide out groups + bias evict."""
                trow = slice(tt * P, (tt + 1) * P)
                yo = opool.tile([P, o_core], F32, name="yo")
                for og in range(o_core // owidth):
                    ocol = slice(og * owidth, (og + 1) * owidth)
                    ps = psum.tile([P, FREE], F32, name="ps")
                    sweep(ps, xhiT, xloT, ocol, owidth)
                    nc.vector.tensor_tensor(
                        out=yo[:, ocol], in0=ps[:, :owidth],
                        in1=bias_bc[:, ocol], op=mybir.AluOpType.add,
                    )
                nc.gpsimd.dma_start(y_d[trow, :], yo)

            # warmup tiles chase W readiness with 128-wide out groups
            prev = prep_tile(0)
            for tt in range(TT):
                if tt + 1 < TT:
                    nxt = prep_tile(tt + 1)
                mm_tile(tt, *prev, P if tt < WARM else FREE)
                if tt + 1 < TT:
                    prev = nxt


def build(t_core=T_CORE, in_dim=IN, o_core=O_CORE):
    nc = bacc.Bacc("TRN2", target_bir_lowering=False, debug=False)
    x_d = nc.dram_tensor("x", [t_core, in_dim], F32, kind="ExternalInput")
    w_d = nc.dram_tensor("w", [o_core, in_dim], F32, kind="ExternalInput")
    b_d = nc.dram_tensor("b", [1, o_core], F32, kind="ExternalInput")
    y_d = nc.dram_tensor("y", [t_core, o_core], F32, kind="ExternalOutput")
    with tile.TileContext(nc) as tc:
        emit(nc, tc, x_d.ap(), w_d.ap(), b_d.ap(), y_d.ap(), t_core, in_dim, o_core)
    nc.compile()
    return nc


_nc_cache = None


def kernel(x: np.ndarray, weight: np.ndarray, bias: np.ndarray, **run_kwargs):
    global _nc_cache
    if _nc_cache is None:
        _nc_cache = build()
    nc = _nc_cache

    x = np.ascontiguousarray(x, dtype=np.float32)
    weight = np.ascontiguousarray(weight, dtype=np.float32)
    bias = np.ascontiguousarray(bias, dtype=np.float32)

    in_maps = []
    for c in range(N_CORES):
        th, oq = divmod(c, O_SPLIT)
        in_maps.append(
            {
                "x": x[th * T_CORE : (th + 1) * T_CORE],
                "w": weight[oq * O_CORE : (oq + 1) * O_CORE],
                "b": bias[oq * O_CORE : (oq + 1) * O_CORE].reshape(1, O_CORE),
            }
        )
    res = run_bass_kernel_spmd(nc, in_maps, core_ids=list(range(N_CORES)), **run_kwargs)
    y = np.empty((TOKENS, OUT), dtype=np.float32)
    for c in range(N_CORES):
        th, oq = divmod(c, O_SPLIT)
        y[th * T_CORE : (th + 1) * T_CORE, oq * O_CORE : (oq + 1) * O_CORE] = (
            res.results[c]["y"]
        )
    kernel.last_results = res
    return y


# revision 35
# speedup vs baseline: 1.0128x; 1.0128x over previous
"""BinaryLinear kernel for Trainium2 (8 NeuronCores, SPMD).

Computes y = x @ sign(W)^T + sign(b) with x:[8192,4096] f32,
W:[4096,4096] f32, b:[4096] f32.

Sharding: 2-way over tokens x 4-way over out_features (8 cores).
Per core: x_shard [4096, 4096], W_shard [1024, 4096], b_shard [1024]
-> y_shard [4096, 1024]. No collectives; host shards/concats.

Math strategy: sign(W) is exactly representable in bf16 (+-1). x is
split into x = hi + lo with hi = bf16(x) and lo = bf16(x - hi);
y = hi @ sW^T + lo @ sW^T accumulated in f32 PSUM reproduces the f32
result to ~2e-6 relative error while the TensorEngine runs at bf16
rate. PE work per core: 4096 LDW+MM pairs (N=512) ~ 874 us;
measured ~1.03 ms on HW (PE ~86% busy).

Structure per core:
  - Phase 0: sign(W)^T built resident in SBUF (8MB bf16) -- W tiles
    loaded in waves of 4 with transfers alternated across the
    ACT-HWDGE and SWDGE paths (parallel transfers), ACT Sign, then one
    batched [128, 4096] -> [128, 32, 128] xbar transpose per tile on
    the SP queue. Bias is broadcast-loaded (stride-0 DMA) and signed.
  - Phase 1 (per 128-token tile): SWDGE x load, DVE hi-cast + lo-sub,
    two xbar transposes into [k, t] layout, 128 MMs into 2 PSUM banks
    (hi sweep then lo sweep per 512-wide out group), DVE bias-add
    eviction, SWDGE store.

Hardware constraints baked into this structure (learned from NTFF
traces and device crashes):
  - A DMA transpose occupies all 16 DMA engines: it is mutually
    exclusive with copy DMAs and pays a ~10us drain when copies are in
    flight. Keep the SP queue transposes-only and serialize phase 0
    cleanly; overlapping x traffic with W prep measures WORSE.
  - Concurrent transposes issued from two HWDGE queues, or matmuls
    racing a transpose into the same SBUF tile, crash the device
    (NRT_EXEC_UNIT_UNRECOVERABLE).
"""

import sys

sys.path.insert(0, "/opt/trn_rl_repo")

import numpy as np

import concourse.bass as bass  # noqa: F401
import concourse.mybir as mybir
from concourse import bacc, tile
from concourse.bass_utils import run_bass_kernel_spmd

TOKENS, IN, OUT = 8192, 4096, 4096
N_CORES = 8
T_SPLIT, O_SPLIT = 2, 4
T_CORE, O_CORE = TOKENS // T_SPLIT, OUT // O_SPLIT

P = 128
FREE = 512  # matmul moving free dim / psum bank width (f32)

F32 = mybir.dt.float32
BF16 = mybir.dt.bfloat16


def emit(nc, tc, x_d, w_d, b_d, y_d, t_core, in_dim, o_core):
    """Emit the per-core program. x_d [t_core, in], w_d [o_core, in],
    b_d [1, o_core], y_d [t_core, o_core]."""
    KS = in_dim // P  # number of 128-wide k slabs
    TT = t_core // P  # token tiles
    OG = o_core // FREE  # 512-wide out groups
    OT = o_core // P  # 128-row tiles of W

    WARM = 0  # tiles that run progressive 128-wide out sweeps (see v9)

    from contextlib import ExitStack

    with ExitStack() as ctx:
        const = ctx.enter_context(tc.tile_pool(name="const", bufs=1))
        # Resident sign(W)^T: [128 k-part, KS slabs, o_core] bf16
        swt = const.tile([P, KS, o_core], BF16)
        bias_bc = const.tile([P, o_core], F32)

        # ---- Phase 0: weights + bias prep ----
        # Waves of 4 full W tiles: loads on ACT HWDGE, signs on ACT,
        # xbar transposes on the SP queue (transposes only).
        with tc.tile_pool(name="wload", bufs=4) as wpool:
            braw = wpool.tile([P, o_core], F32, name="braw", bufs=1)
            nc.gpsimd.dma_start(braw, b_d.to_broadcast([P, o_core]))
            nc.scalar.sign(bias_bc, braw)
            for w0 in range(0, OT, 4):
                wfs, wss = [], []
                for ot in range(w0, min(w0 + 4, OT)):
                    wf = wpool.tile([P, in_dim], F32, name="wf")
                    eng = nc.scalar if ot % 2 == 0 else nc.gpsimd
                    eng.dma_start(wf, w_d[ot * P : (ot + 1) * P, :])
                    wfs.append(wf)
                for wf in wfs:
                    ws = wpool.tile([P, in_dim], BF16, name="ws")
                    nc.scalar.sign(ws, wf)  # +-1 in bf16
                    wss.append(ws)
                for ot, ws in zip(range(w0, w0 + 4), wss):
                    # [128 o, in] -> [128 k, KS, 128 o]
                    nc.sync.dma_start_transpose(
                        swt[:, :, ot * P : (ot + 1) * P], ws
                    )

        # ---- Phase 1 ----
        with (
            tc.tile_pool(name="xload", bufs=2) as xpool,
            tc.tile_pool(name="hilo", bufs=2) as hpool,
            tc.tile_pool(name="xt", bufs=3) as tpool,
            tc.tile_pool(name="psum", bufs=8, space="PSUM") as psum,
            tc.tile_pool(name="yout", bufs=3) as opool,
        ):

            def prep_tile(tt):
                """x f32 load -> hi cast + lo sub (DVE) -> xbar transposes."""
                trow = slice(tt * P, (tt + 1) * P)
                xf = xpool.tile([P, in_dim], F32, name="xf")
                nc.gpsimd.dma_start(xf, x_d[trow, :])
                xhi = hpool.tile([P, in_dim], BF16, name="xhi")
                nc.vector.tensor_copy(out=xhi, in_=xf)
                xhiT = tpool.tile([P, KS, P], BF16, name="xhiT")
                nc.sync.dma_start_transpose(xhiT, xhi)
                xlo = hpool.tile([P, in_dim], BF16, name="xlo")
                nc.vector.tensor_tensor(
                    out=xlo, in0=xf, in1=xhi, op=mybir.AluOpType.subtract
                )
                xloT = tpool.tile([P, KS, P], BF16, name="xloT")
                nc.sync.dma_start_transpose(xloT, xlo)
                return xhiT, xloT

            def sweep(ps, xhiT, xloT, ocol, width):
                for ks in range(KS):
                    nc.tensor.matmul(
                        ps[:, :width], xhiT[:, ks, :], swt[:, ks, ocol],
                        start=(ks == 0), stop=False,
                    )
                for ks in range(KS):
                    nc.tensor.matmul(
                        ps[:, :width], xloT[:, ks, :], swt[:, ks, ocol],
                        start=False, stop=(ks == KS - 1),
                    )

            def mm_tile(tt, xhiT, xloT, owidth):
                """matmul sweeps in owidth-wide out groups + bias evict."""
                trow = slice(tt * P, (tt + 1) * P)
                yo = opool.tile([P, o_core], F32, name="yo")
                for og in range(o_core // owidth):
                    ocol = slice(og * owidth, (og + 1) * owidth)
                    ps = psum.tile([P, FREE], F32, name="ps")
                    sweep(ps, xhiT, xloT, ocol, owidth)
                    nc.vector.tensor_tensor(
                        out=yo[:, ocol], in0=ps[:, :owidth],
                        in1=bias_bc[:, ocol], op=mybir.AluOpType.add,
                    )
                nc.gpsimd.dma_start(y_d[trow, :], yo)

            # warmup tiles chase W readiness with 128-wide out groups
            prev = prep_tile(0)
            for tt in range(TT):
                if tt + 1 < TT:
                    nxt = prep_tile(tt + 1)
                mm_tile(tt, *prev, P if tt < WARM else FREE)
                if tt + 1 < TT:
                    prev = nxt


def build(t_core=T_CORE, in_dim=IN, o_core=O_CORE):
    nc = bacc.Bacc("TRN2", target_bir_lowering=False, debug=False)
    x_d = nc.dram_tensor("x", [t_core, in_dim], F32, kind="ExternalInput")
    w_d = nc.dram_tensor("w", [o_core, in_dim], F32, kind="ExternalInput")
    b_d = nc.dram_tensor("b", [1, o_core], F32, kind="ExternalInput")
    y_d = nc.dram_tensor("y", [t_core, o_core], F32, kind="ExternalOutput")
    with tile.TileContext(nc) as tc:
        emit(nc, tc, x_d.ap(), w_d.ap(), b_d.ap(), y_d.ap(), t_core, in_dim, o_core)
    nc.compile()
    return nc


_nc_cache = None


def kernel(x: np.ndarray, weight: np.ndarray, bias: np.ndarray, **run_kwargs):
    global _nc_cache
    if _nc_cache is None:
        _nc_cache = build()
    nc = _nc_cache

    x = np.ascontiguousarray(x, dtype=np.float32)
    weight = np.ascontiguousarray(weight, dtype=np.float32)
    bias = np.ascontiguousarray(bias, dtype=np.float32)

    in_maps = []
    for c in range(N_CORES):
        th, oq = divmod(c, O_SPLIT)
        in_maps.append(
            {
                "x": x[th * T_CORE : (th + 1) * T_CORE],
                "w": weight[oq * O_CORE : (oq + 1) * O_CORE],
                "b": bias[oq * O_CORE : (oq + 1) * O_CORE].reshape(1, O_CORE),
            }
        )
    res = run_bass_kernel_spmd(nc, in_maps, core_ids=list(range(N_CORES)), **run_kwargs)
    y = np.empty((TOKENS, OUT), dtype=np.float32)
    for c in range(N_CORES):
        th, oq = divmod(c, O_SPLIT)
        y[th * T_CORE : (th + 1) * T_CORE, oq * O_CORE : (oq + 1) * O_CORE] = (
            res.results[c]["y"]
        )
    kernel.last_results = res
    return y
